# revision 69
# baseline (speedup 1.0000x reference)
"""KGE (TransR-style) loss kernel for Trainium2, 8 NeuronCores.

Strategy:
  - Host: sort the M=8192 triples by relation id (pure index manipulation),
    pad each relation's segment to 128-row blocks -> ~96 single-relation
    blocks, distributed evenly across the 8 cores (same block count per
    core, so one SPMD program serves all cores). Per-core relation tables
    (W blocks, r rows) are sharded host-side per the block list.
  - Device (per core, per block b):
      * three indirect DMAs gather the h/pos/neg entity rows into
        X = [H | P | N]  (128 x 384)   [GPSIMD/SWDGE]
      * D_pos = H - P, D_neg = H - N; squares + row reductions  [DVE]
      * PE transpose D -> D^T; ACT copies PSUM->SBUF
      * matmul D^T.T @ W_b accumulated with a K=NB one-hot matmul adding
        r_b -> (h - t) @ W + r in PSUM  [PE]
      * score diff col stored per block; softplus tail batched over all
        blocks at the end (2 act-table loads total instead of ~2/block)
  - reg = 0.5*sum(X^2) per row, masked+scaled by 1e-5 via the wval input;
    relation-embedding reg via per-block counts.
  - Final: free-dim reduce + ones-matmul partition reduce -> one f32 per
    core; host sums the 8 partials and divides by M.
"""

import os
from contextlib import ExitStack

import numpy as np

import concourse.bass as bass
import concourse.tile as tile
from concourse import bacc, mybir
from concourse.masks import make_identity

M = 8192
E = 128
N_ENT = 500000
N_REL = 64
LAM = 1e-5
P = 128
N_CORES = 8
PAD_BIAS = -30000.0

f32 = mybir.dt.float32
i32 = mybir.dt.int32

_cache = {}


def _build(NB: int):
    """Build + compile the single-core SPMD program for NB blocks/core."""
    nc = bacc.Bacc(
        "TRN2",
        target_bir_lowering=False,
        debug=False,
        num_devices=N_CORES,
    )

    ent = nc.dram_tensor("ent", (N_ENT, E), f32, kind="ExternalInput").ap()
    idx3 = nc.dram_tensor("idx3", (P, NB * 3), i32, kind="ExternalInput").ap()
    mbias = nc.dram_tensor("mbias", (P, NB), f32, kind="ExternalInput").ap()
    wval = nc.dram_tensor("wval", (P, NB), f32, kind="ExternalInput").ap()
    w_all = nc.dram_tensor("w_all", (P, NB * P), f32, kind="ExternalInput").ap()
    r_in = nc.dram_tensor("r_blk", (NB, E), f32, kind="ExternalInput").ap()
    lsel = nc.dram_tensor("lsel", (NB, NB * P), f32, kind="ExternalInput").ap()
    cnt = nc.dram_tensor("cnt", (NB, 1), f32, kind="ExternalInput").ap()
    out = nc.dram_tensor("out", (1, 1), f32, kind="ExternalOutput").ap()

    with tile.TileContext(nc) as tc, ExitStack() as ctx:
        const = ctx.enter_context(tc.tile_pool(name="const", bufs=1))
        xp = ctx.enter_context(tc.tile_pool(name="xp", bufs=6))
        dp = ctx.enter_context(tc.tile_pool(name="dp", bufs=3))
        dtp = ctx.enter_context(tc.tile_pool(name="dtp", bufs=3))
        scrp = ctx.enter_context(tc.tile_pool(name="scrp", bufs=3))
        colp = ctx.enter_context(tc.tile_pool(name="colp", bufs=4))
        ps_t = ctx.enter_context(tc.tile_pool(name="ps_t", bufs=2, space="PSUM"))
        ps_mm = ctx.enter_context(tc.tile_pool(name="ps_mm", bufs=2, space="PSUM"))

        # constants / small inputs
        iden = const.tile([P, P], f32)
        make_identity(nc, iden[:])
        ones_col = const.tile([P, 1], f32)
        nc.gpsimd.memset(ones_col[:], 1.0)

        idx3_sb = const.tile([P, NB * 3], i32)
        nc.sync.dma_start(out=idx3_sb[:], in_=idx3[:])
        mb_sb = const.tile([P, NB], f32)
        nc.sync.dma_start(out=mb_sb[:], in_=mbias[:])
        wv_sb = const.tile([P, NB], f32)
        nc.sync.dma_start(out=wv_sb[:], in_=wval[:])
        cnt_sb = const.tile([NB, 1], f32)
        nc.sync.dma_start(out=cnt_sb[:], in_=cnt[:])
        w_sb = const.tile([P, NB * P], f32)
        nc.sync.dma_start(out=w_sb[:], in_=w_all[:])
        r_blk = const.tile([NB, E], f32)
        nc.sync.dma_start(out=r_blk[:], in_=r_in[:])
        lsel_sb = const.tile([NB, NB * P], f32)
        nc.sync.dma_start(out=lsel_sb[:], in_=lsel[:])

        # per-block score-diff columns and raw reg columns
        dcols = const.tile([P, NB], f32)
        regs = const.tile([P, NB], f32)

        for b in range(NB):
            # three gathers: hardware indirect DMA takes one index per
            # partition and reads out.free_size contiguous elems from it
            x = xp.tile([P, 3 * E], f32, tag="x")
            for j in range(3):
                nc.gpsimd.indirect_dma_start(
                    out=x[:, j * E : (j + 1) * E],
                    out_offset=None,
                    in_=ent[:],
                    in_offset=bass.IndirectOffsetOnAxis(
                        ap=idx3_sb[:, 3 * b + j : 3 * b + j + 1], axis=0
                    ),
                )

            # raw reg col: sum over [H|P|N] of squares (mask+scale at tail);
            # ACT Square with accum_out frees the DVE for score work
            xsq = scrp.tile([P, 3 * E], f32, tag="xsq")
            nc.scalar.activation(
                out=xsq[:], in_=x[:],
                func=mybir.ActivationFunctionType.Square,
                accum_out=regs[:, b : b + 1],
            )

            # D_pos = H - P, D_neg = H - N
            d_pos = dp.tile([P, E], f32, tag="dpos")
            nc.vector.tensor_tensor(
                out=d_pos[:], in0=x[:, 0:E], in1=x[:, E : 2 * E],
                op=mybir.AluOpType.subtract,
            )
            d_neg = dp.tile([P, E], f32, tag="dneg")
            nc.vector.tensor_tensor(
                out=d_neg[:], in0=x[:, 0:E], in1=x[:, 2 * E : 3 * E],
                op=mybir.AluOpType.subtract,
            )

            # transpose D -> D^T (PSUM), copy to SBUF on ACT
            dpt_ps = ps_t.tile([P, P], f32, tag="tp")
            nc.tensor.transpose(out=dpt_ps[:], in_=d_pos[:], identity=iden[:])
            dnt_ps = ps_t.tile([P, P], f32, tag="tn")
            nc.tensor.transpose(out=dnt_ps[:], in_=d_neg[:], identity=iden[:])
            dpt = dtp.tile([P, P], f32, tag="dpt")
            nc.scalar.copy(dpt[:], dpt_ps[:])
            dnt = dtp.tile([P, P], f32, tag="dnt")
            nc.scalar.copy(dnt[:], dnt_ps[:])

            # (h - t) @ W + r
            wb = w_sb[:, b * P : (b + 1) * P]
            lb = lsel_sb[:, b * P : (b + 1) * P]
            pos_ps = ps_mm.tile([P, E], f32, tag="mp")
            nc.tensor.matmul(out=pos_ps[:], lhsT=dpt[:], rhs=wb, start=True, stop=False)
            nc.tensor.matmul(out=pos_ps[:], lhsT=lb, rhs=r_blk[:], start=False, stop=True)
            neg_ps = ps_mm.tile([P, E], f32, tag="mn")
            nc.tensor.matmul(out=neg_ps[:], lhsT=dnt[:], rhs=wb, start=True, stop=False)
            nc.tensor.matmul(out=neg_ps[:], lhsT=lb, rhs=r_blk[:], start=False, stop=True)

            # score diff col (x2): sum(neg^2) - sum(pos^2); ACT Square reads
            # PSUM (DVE cannot read two PSUM inputs) and fuses the reduction
            psq = scrp.tile([P, E], f32, tag="psq")
            spos = colp.tile([P, 1], f32, tag="sp")
            nc.scalar.activation(
                out=psq[:], in_=pos_ps[:],
                func=mybir.ActivationFunctionType.Square,
                accum_out=spos[:],
            )
            nsq = scrp.tile([P, E], f32, tag="nsq")
            sneg = colp.tile([P, 1], f32, tag="sn")
            nc.scalar.activation(
                out=nsq[:], in_=neg_ps[:],
                func=mybir.ActivationFunctionType.Square,
                accum_out=sneg[:],
            )
            nc.vector.tensor_tensor(
                out=dcols[:, b : b + 1], in0=sneg[:], in1=spos[:],
                op=mybir.AluOpType.subtract,
            )

        # ---- batched tail over all NB blocks ----
        # loss = softplus(0.5*dcols + mbias) = relu(y) + ln(1 + exp(-|y|))
        dm = const.tile([P, NB], f32)
        nc.vector.tensor_scalar_mul(out=dm[:], in0=dcols[:], scalar1=0.5)
        nc.vector.tensor_tensor(
            out=dm[:], in0=dm[:], in1=mb_sb[:], op=mybir.AluOpType.add
        )
        t_abs = const.tile([P, NB], f32)
        nc.scalar.activation(
            out=t_abs[:], in_=dm[:], func=mybir.ActivationFunctionType.Abs
        )
        t_exp = const.tile([P, NB], f32)
        nc.scalar.activation(
            out=t_exp[:], in_=t_abs[:], func=mybir.ActivationFunctionType.Exp,
            scale=-1.0,
        )
        t_ln = const.tile([P, NB], f32)
        nc.scalar.activation(
            out=t_ln[:], in_=t_exp[:], func=mybir.ActivationFunctionType.Ln,
            bias=1.0,
        )
        t_relu = const.tile([P, NB], f32)
        nc.scalar.activation(
            out=t_relu[:], in_=dm[:], func=mybir.ActivationFunctionType.Relu
        )

        acc = const.tile([P, 2 * NB], f32)
        nc.vector.tensor_tensor(
            out=acc[:, :NB], in0=t_ln[:], in1=t_relu[:], op=mybir.AluOpType.add
        )
        # reg masked + scaled (wval holds 0.5*1e-5 or 0)
        nc.vector.tensor_tensor(
            out=acc[:, NB:], in0=regs[:], in1=wv_sb[:], op=mybir.AluOpType.mult
        )

        # relation-embedding reg: cnt_b * 0.5*||r_b||^2 (cnt pre-scaled 1e-5)
        rsq = const.tile([NB, E], f32)
        nc.vector.tensor_tensor(
            out=rsq[:], in0=r_blk[:], in1=r_blk[:], op=mybir.AluOpType.mult
        )
        rr_col = const.tile([NB, 1], f32)
        nc.vector.reduce_sum(out=rr_col[:], in_=rsq[:], axis=mybir.AxisListType.X)
        rr_s = const.tile([NB, 1], f32)
        nc.vector.tensor_tensor(
            out=rr_s[:], in0=rr_col[:], in1=cnt_sb[:], op=mybir.AluOpType.mult
        )

        # total per-partition, then partition-reduce via ones matmul
        t_all = const.tile([P, 1], f32)
        nc.vector.reduce_sum(out=t_all[:], in_=acc[:], axis=mybir.AxisListType.X)
        nc.vector.tensor_tensor(
            out=t_all[:NB], in0=t_all[:NB], in1=rr_s[:], op=mybir.AluOpType.add
        )
        fin_ps = ps_mm.tile([1, 1], f32, tag="mp")
        nc.tensor.matmul(out=fin_ps[:], lhsT=t_all[:], rhs=ones_col[:], start=True, stop=True)
        fin_sb = const.tile([1, 1], f32)
        nc.scalar.copy(fin_sb[:], fin_ps[:])
        nc.sync.dma_start(out=out[:], in_=fin_sb[:])

    nc.compile()
    return nc


def _plan(h, r, pos_t, neg_t, relation_weight, relation_embed):
    """Sort by relation, pad to 128-row single-relation blocks, split 8 ways."""
    order = np.argsort(r, kind="stable")
    counts = np.bincount(r, minlength=N_REL)
    blocks = []
    pos = 0
    for k in range(N_REL):
        c = int(counts[k])
        ids = order[pos : pos + c]
        pos += c
        for s in range(0, c, P):
            blocks.append((k, ids[s : s + P]))
    nb = max(2, -(-len(blocks) // N_CORES))
    while len(blocks) < nb * N_CORES:
        blocks.append((0, np.empty(0, np.int64)))

    maps = []
    for c in range(N_CORES):
        core_blocks = blocks[c * nb : (c + 1) * nb]
        idx3 = np.zeros((P, nb, 3), np.int32)
        mb = np.full((P, nb), PAD_BIAS, np.float32)
        wv = np.zeros((P, nb), np.float32)
        cnt = np.zeros((nb, 1), np.float32)
        w_blk = np.zeros((P, nb, P), np.float32)
        r_blk = np.zeros((nb, E), np.float32)
        for b, (k, ids) in enumerate(core_blocks):
            n = len(ids)
            if n:
                idx3[:n, b, 0] = h[ids]
                idx3[:n, b, 1] = pos_t[ids]
                idx3[:n, b, 2] = neg_t[ids]
            mb[:n, b] = 0.0
            wv[:n, b] = 0.5 * LAM
            cnt[b, 0] = n * LAM
            w_blk[:, b, :] = relation_weight[k]
            r_blk[b, :] = relation_embed[k]
        maps.append(
            {
                "idx3": idx3.reshape(P, nb * 3),
                "mbias": mb,
                "wval": wv,
                "cnt": cnt,
                "w_all": np.ascontiguousarray(w_blk.reshape(P, nb * P)),
                "r_blk": r_blk,
                "lsel": np.kron(np.eye(nb, dtype=np.float32), np.ones((1, P), np.float32)),
            }
        )
    return nb, maps


def kernel(h, r, pos_t, neg_t, entity_embed, relation_embed, relation_weight):
    h = np.asarray(h).astype(np.int32)
    r = np.asarray(r).astype(np.int32)
    pos_t = np.asarray(pos_t).astype(np.int32)
    neg_t = np.asarray(neg_t).astype(np.int32)
    ent = np.ascontiguousarray(np.asarray(entity_embed, dtype=np.float32))
    re = np.ascontiguousarray(np.asarray(relation_embed, dtype=np.float32))
    rw = np.ascontiguousarray(np.asarray(relation_weight, dtype=np.float32))

    nb, maps = _plan(h, r, pos_t, neg_t, rw, re)
    if nb not in _cache:
        _cache[nb] = _build(nb)
    nc = _cache[nb]

    in_maps = [{"ent": ent, **maps[c]} for c in range(N_CORES)]

    if os.environ.get("KGE_SIM"):
        from concourse.bass_interp import CoreSim

        total = 0.0
        for c in range(N_CORES):
            sim = CoreSim(nc, trace=False)
            for name, arr in in_maps[c].items():
                sim.tensor(name)[:] = arr
            sim.simulate()
            total += float(sim.tensor("out")[0, 0])
        return np.float32(total / M)

    from concourse.bass_utils import run_bass_kernel_spmd

    res = run_bass_kernel_spmd(nc, in_maps, core_ids=list(range(N_CORES)))
    total = sum(float(res.results[c]["out"][0, 0]) for c in range(N_CORES))
    return np.float32(total / M)

